# revision 10
# baseline (speedup 1.0000x reference)
"""MoE-routed K-cluster autoencoder kernel for 8 Trainium2 NeuronCores.

Strategy
--------
Each row of x is reconstructed by the autoencoder of its kmeans cluster.
Computing all K experts densely for every row (like the reference) does
10x the needed matmul work, so we *route*:

  host:   sort rows by cluster, pack them into fixed-capacity "slots"
          (one cluster per slot; 4 slots x 1280 rows per core for uniform
          labels), pre-transpose so features lie on SBUF partitions, and
          flatten each row-chunk k-major so every DMA moves long
          contiguous runs.
  device: per slot, run the 6-layer MLP chain as feature-major matmuls
          (outT = W.T @ actsT) in fp32r (full-rate fp32) on the PE.
          Weight-stationary phase order: each weight tile is loaded once
          and streams all of the slot's row-chunks back-to-back, so the
          PE stays dense (HAM-warm) and layer-boundary waits are covered
          by the other chunks' matmuls. Bias+ReLU fuse into ScalarE
          activations that also evict PSUM->SBUF; the last layer's bias
          rides on VectorE.
  host:   scatter the per-slot outputs back to original row order.

All shapes are static; the slot capacity config adapts to the label
histogram (uniform labels always give the (4, 1280) config).
"""

import numpy as np

import concourse.tile as tile
from concourse import bacc, mybir
from concourse.bass_utils import run_bass_kernel_spmd

N_CORES = 8
B, D, H1, H2, L, K = 32768, 784, 256, 64, 16, 10
P = 112          # partition tile height for the D axis: 784 = 7 * 112
KT = D // P      # 7 k-tiles along D

# per-slot packed weight layout (column offsets in a [128, WSLOT] block)
_E0, _E1, _E2, _D0, _D1, _D2 = 0, 1792, 1920, 1936, 2000, 2256
WSLOT = 3824     # = 7*256 + 2*64 + 16 + 64 + 256 + 2*784
BSLOT = 14       # bias columns per slot: 2 + 1 + 1 + 1 + 2 + 7

# (slots_per_core, rows_per_slot)
_CONFIGS = [(4, 1152), (4, 1280), (8, 640), (16, 320), (32, 160)]

_F32 = mybir.dt.float32
_F32R = mybir.dt.float32r
_BF16 = mybir.dt.bfloat16
_RELU = mybir.ActivationFunctionType.Relu

# matmul-operand dtype mode: "f32r" (precise, fp32 bytes on the wire) or
# "bf16" (half the x/weight DMA bytes, fast weight loads, ~7e-3 rel err)
MODE = "f32r"


def _mdt_view(ap, mode):
    return ap.bitcast(_F32R) if mode == "f32r" else ap


def _chunks(R, mode="f32r"):
    """Split R rows into moving-operand chunks <=512.

    f32r: each chunk >=256 (full-rate threshold), so rebalance the tail.
    bf16: plain greedy 512s -- chunk starts must stay PSUM-bank aligned
    because matmuls write slices of one merged multi-bank psum tile."""
    out, rem = [], R
    while rem > 0:
        c = min(512, rem)
        if mode == "f32r" and c == 512 and 0 < rem - c < 256:
            c = max(256, min(512, (rem + 1) // 2))
        out.append(c)
        rem -= c
    return out


def _build_program(S, R, mode):
    mdt = _F32R if mode == "f32r" else _BF16
    idt = _F32 if mode == "f32r" else _BF16
    ncols = S * R
    nflat = ncols * KT  # x/y are stored chunk-flattened: [P, sum(KT*nch)]
    nc = bacc.Bacc("TRN2", target_bir_lowering=False, debug=False)
    xt = nc.dram_tensor("xt", [P, nflat], idt, kind="ExternalInput").ap()
    wp = nc.dram_tensor("wp", [128, S * WSLOT], idt, kind="ExternalInput").ap()
    bp = nc.dram_tensor("bp", [128, S * BSLOT], _F32, kind="ExternalInput").ap()
    yt = nc.dram_tensor("yt", [P, nflat], idt, kind="ExternalOutput").ap()

    chunks = _chunks(R, mode)
    NCH = len(chunks)

    with tile.TileContext(nc) as tc:
        with (
            tc.tile_pool(name="wpool", bufs=1) as wpool,
            tc.tile_pool(name="iopool", bufs=1) as iopool,
            tc.tile_pool(name="apool", bufs=1) as apool,
            tc.tile_pool(name="pspool", bufs=1, space="PSUM") as pspool,
        ):
            bsb = wpool.tile([128, S * BSLOT], _F32, tag="b", name="bsb", bufs=1)
            nc.sync.dma_start(out=bsb, in_=bp)

            # PE pre-warm: ~3.5us of throwaway matmuls on a zeroed tile so
            # the HAM clock-gate opens to 2.4 GHz while the first DMAs land.
            wu = wpool.tile([128, 512], mdt, tag="wu", name="wu", bufs=1)
            nc.vector.memset(wu, 0)
            wups = pspool.tile([128, 512], _F32, tag="ps", name="wups", bufs=8)
            for _ in range(18):
                nc.tensor.matmul(wups, wu[:, 0:128], wu, start=True, stop=True)

            def bias(lo, col):
                return bsb[0:lo, col:col + 1]

            def ps_tile(parts, nch):
                return pspool.tile([parts, nch], _F32, tag="ps", name="ps",
                                   bufs=8)

            for s in range(S):
                w = wpool.tile([128, WSLOT], mdt, tag="w", name="w", bufs=2)
                nc.sync.dma_start(
                    out=w, in_=_mdt_view(wp[:, s * WSLOT:(s + 1) * WSLOT], mode))
                bb = s * BSLOT

                # chunk-flattened column offsets for this slot
                offs = []
                cum = s * R * KT
                for nch in chunks:
                    offs.append(cum)
                    cum += nch * KT

                xq = []
                for ci, nch in enumerate(chunks):
                    t = iopool.tile([128, KT, nch], mdt, tag="xq", name="xq",
                                    bufs=NCH + (3 if mode == "bf16" else 2))
                    nc.sync.dma_start(
                        out=t[0:P],
                        in_=_mdt_view(xt[:, offs[ci]:offs[ci] + KT * nch]
                        .rearrange("p (k n) -> p k n", k=KT), mode))
                    xq.append(t)

                # encoder 0: [784 -> 256]; weight-stationary over chunks
                h1 = [[None] * NCH, [None] * NCH]
                for m in range(2):
                    ps = [None] * NCH
                    for k in range(KT):
                        wk = w[0:P, _E0 + k * 256 + 128 * m:
                               _E0 + k * 256 + 128 * m + 128]
                        for ci, nch in enumerate(chunks):
                            if k == 0:
                                ps[ci] = ps_tile(128, nch)
                            nc.tensor.matmul(ps[ci], wk, xq[ci][0:P, k, :],
                                             start=(k == 0), stop=(k == KT - 1))
                    for ci, nch in enumerate(chunks):
                        t = apool.tile([128, nch], mdt, tag="h1", name="h1",
                                       bufs=7)
                        nc.scalar.activation(t, ps[ci], _RELU,
                                             bias=bias(128, bb + m))
                        h1[m][ci] = t

                # encoder 1: [256 -> 64]
                ps = [None] * NCH
                for k in range(2):
                    wk = w[0:128, _E1 + 64 * k:_E1 + 64 * k + 64]
                    for ci, nch in enumerate(chunks):
                        if k == 0:
                            ps[ci] = ps_tile(64, nch)
                        nc.tensor.matmul(ps[ci], wk, h1[k][ci],
                                         start=(k == 0), stop=(k == 1))
                h2 = []
                for ci, nch in enumerate(chunks):
                    t = apool.tile([64, nch], mdt, tag="h2", name="h2",
                                   bufs=4)
                    nc.scalar.activation(t, ps[ci], _RELU, bias=bias(64, bb + 2))
                    h2.append(t)

                # encoder 2: [64 -> 16]
                ps = [None] * NCH
                wk = w[0:64, _E2:_E2 + 16]
                for ci, nch in enumerate(chunks):
                    ps[ci] = ps_tile(16, nch)
                    nc.tensor.matmul(ps[ci], wk, h2[ci], start=True, stop=True)
                z = []
                for ci, nch in enumerate(chunks):
                    t = apool.tile([16, nch], mdt, tag="z", name="z",
                                   bufs=4)
                    nc.scalar.activation(t, ps[ci], _RELU, bias=bias(16, bb + 3))
                    z.append(t)

                # decoder 0: [16 -> 64]
                ps = [None] * NCH
                wk = w[0:16, _D0:_D0 + 64]
                for ci, nch in enumerate(chunks):
                    ps[ci] = ps_tile(64, nch)
                    nc.tensor.matmul(ps[ci], wk, z[ci], start=True, stop=True)
                a1 = []
                for ci, nch in enumerate(chunks):
                    t = apool.tile([64, nch], mdt, tag="a1", name="a1",
                                   bufs=4)
                    nc.scalar.activation(t, ps[ci], _RELU, bias=bias(64, bb + 4))
                    a1.append(t)

                # decoder 1: [64 -> 256]
                a2 = [[None] * NCH, [None] * NCH]
                for m in range(2):
                    wk = w[0:64, _D1 + 128 * m:_D1 + 128 * m + 128]
                    ps = [None] * NCH
                    for ci, nch in enumerate(chunks):
                        ps[ci] = ps_tile(128, nch)
                        nc.tensor.matmul(ps[ci], wk, a1[ci], start=True, stop=True)
                    for ci, nch in enumerate(chunks):
                        t = apool.tile([128, nch], mdt, tag="a2", name="a2",
                                       bufs=7)
                        nc.scalar.activation(t, ps[ci], _RELU,
                                             bias=bias(128, bb + 5 + m))
                        a2[m][ci] = t

                # decoder 2: [256 -> 784], bias only (VectorE)
                yq = []
                for ci, nch in enumerate(chunks):
                    yq.append(iopool.tile([128, KT, nch], idt, tag="yq",
                                          name="yq", bufs=NCH + 1))
                for mm in range(KT):
                    ps = [None] * NCH
                    for k in range(2):
                        wk = w[0:128, _D2 + 784 * k + 112 * mm:
                               _D2 + 784 * k + 112 * mm + 112]
                        for ci, nch in enumerate(chunks):
                            if k == 0:
                                ps[ci] = ps_tile(112, nch)
                            nc.tensor.matmul(ps[ci], wk, a2[k][ci],
                                             start=(k == 0), stop=(k == 1))
                    for ci, nch in enumerate(chunks):
                        nc.vector.tensor_scalar_add(
                            yq[ci][0:P, mm, :], ps[ci], bias(112, bb + 7 + mm))
                for ci, nch in enumerate(chunks):
                    nc.sync.dma_start(
                        out=yt[:, offs[ci]:offs[ci] + KT * nch]
                        .rearrange("p (k n) -> p k n", k=KT),
                        in_=yq[ci][0:P])
    nc.compile()
    return nc




def _build_program_v3(S, R):
    """bf16 path: slot-level x/y tiles (one DMA each), merged multi-bank
    PSUM accumulator per weight group (one drain op instead of one per
    chunk), drains balanced across ScalarE/VectorE, bf16 output writes."""
    ncols = S * R
    nflat = ncols * KT
    nc = bacc.Bacc("TRN2", target_bir_lowering=False, debug=False)
    xt = nc.dram_tensor("xt", [P, nflat], _BF16, kind="ExternalInput").ap()
    wp = nc.dram_tensor("wp", [128, S * WSLOT], _BF16, kind="ExternalInput").ap()
    bp = nc.dram_tensor("bp", [128, S * BSLOT], _F32, kind="ExternalInput").ap()
    yt = nc.dram_tensor("yt", [P, nflat], _BF16, kind="ExternalOutput").ap()

    chunks = _chunks(R, "bf16")
    slices = []
    off = 0
    for nch in chunks:
        slices.append((off, nch))
        off += nch

    with tile.TileContext(nc) as tc:
        with (
            tc.tile_pool(name="wpool", bufs=1) as wpool,
            tc.tile_pool(name="iopool", bufs=1) as iopool,
            tc.tile_pool(name="apool", bufs=1) as apool,
            tc.tile_pool(name="pspool", bufs=1, space="PSUM") as pspool,
        ):
            bsb = wpool.tile([128, S * BSLOT], _F32, tag="b", name="bsb", bufs=1)
            nc.sync.dma_start(out=bsb, in_=bp)

            # PE pre-warm: ~3.5us of throwaway matmuls on a zeroed tile so
            # the HAM clock-gate opens to 2.4 GHz while the first DMAs land.
            wu = wpool.tile([128, 512], mdt, tag="wu", name="wu", bufs=1)
            nc.vector.memset(wu, 0)
            wups = pspool.tile([128, 512], _F32, tag="ps", name="wups", bufs=8)
            for _ in range(18):
                nc.tensor.matmul(wups, wu[:, 0:128], wu, start=True, stop=True)

            def bias(lo, col):
                return bsb[0:lo, col:col + 1]

            def mm_group(ps_parts, w_tiles, rhs_of):
                """One merged accumulator: ps[:, off:off+nch] accumulates
                w_tiles[k].T @ rhs_of(k)[:, off:off+nch] over k."""
                ps = pspool.tile([ps_parts, R], _F32, tag="ps", name="ps",
                                 bufs=2)
                nk = len(w_tiles)
                for k in range(nk):
                    for off, nch in slices:
                        nc.tensor.matmul(ps[:, off:off + nch], w_tiles[k],
                                         rhs_of(k)[:, off:off + nch],
                                         start=(k == 0), stop=(k == nk - 1))
                return ps

            for s in range(S):
                w = wpool.tile([128, WSLOT], _BF16, tag="w", name="w", bufs=2)
                nc.sync.dma_start(out=w, in_=wp[:, s * WSLOT:(s + 1) * WSLOT])
                bb = s * BSLOT

                xq = iopool.tile([128, KT, R], _BF16, tag="xq", name="xq",
                                 bufs=3)
                nc.sync.dma_start(
                    out=xq[0:P],
                    in_=xt[:, s * R * KT:(s + 1) * R * KT]
                    .rearrange("p (k n) -> p k n", k=KT))

                # encoder 0: [784 -> 256]
                h1 = []
                for m in range(2):
                    ps = mm_group(
                        128,
                        [w[0:P, _E0 + k * 256 + 128 * m:
                           _E0 + k * 256 + 128 * m + 128] for k in range(KT)],
                        lambda k: xq[0:P, k, :])
                    t = apool.tile([128, R], _BF16, tag="h1", name="h1", bufs=3)
                    nc.scalar.activation(t, ps, _RELU, bias=bias(128, bb + m))
                    h1.append(t)

                # encoder 1: [256 -> 64]
                ps = mm_group(64, [w[0:128, _E1 + 64 * k:_E1 + 64 * k + 64]
                                   for k in range(2)], lambda k: h1[k])
                h2 = apool.tile([64, R], _BF16, tag="h2", name="h2", bufs=2)
                nc.scalar.activation(h2, ps, _RELU, bias=bias(64, bb + 2))

                # encoder 2: [64 -> 16]
                ps = mm_group(16, [w[0:64, _E2:_E2 + 16]], lambda k: h2)
                z = apool.tile([16, R], _BF16, tag="z", name="z", bufs=2)
                nc.scalar.activation(z, ps, _RELU, bias=bias(16, bb + 3))

                # decoder 0: [16 -> 64]
                ps = mm_group(64, [w[0:16, _D0:_D0 + 64]], lambda k: z)
                a1 = apool.tile([64, R], _BF16, tag="a1", name="a1", bufs=2)
                nc.scalar.activation(a1, ps, _RELU, bias=bias(64, bb + 4))

                # decoder 1: [64 -> 256]
                a2 = []
                for m in range(2):
                    ps = mm_group(128, [w[0:64, _D1 + 128 * m:
                                          _D1 + 128 * m + 128]], lambda k: a1)
                    t = apool.tile([128, R], _BF16, tag="a2", name="a2", bufs=3)
                    nc.scalar.activation(t, ps, _RELU,
                                         bias=bias(128, bb + 5 + m))
                    a2.append(t)

                # decoder 2: [256 -> 784], bias only; balance ACT vs DVE
                yq = iopool.tile([128, KT, R], _BF16, tag="yq", name="yq",
                                 bufs=2)
                for mm in range(KT):
                    ps = mm_group(
                        112,
                        [w[0:128, _D2 + 784 * k + 112 * mm:
                           _D2 + 784 * k + 112 * mm + 112] for k in range(2)],
                        lambda k: a2[k])
                    if mm < 2:
                        nc.scalar.add(yq[0:P, mm, :], ps, bias(112, bb + 7 + mm))
                    else:
                        nc.vector.tensor_scalar_add(
                            yq[0:P, mm, :], ps, bias(112, bb + 7 + mm))
                nc.sync.dma_start(
                    out=yt[:, s * R * KT:(s + 1) * R * KT]
                    .rearrange("p (k n) -> p k n", k=KT),
                    in_=yq[0:P])
    nc.compile()
    return nc


_programs = {}


def _get_program(S, R, mode):
    if (S, R, mode) not in _programs:
        _programs[(S, R, mode)] = _build_program(S, R, mode)
    return _programs[(S, R, mode)]


def _pack_weights(params, slot_clusters):
    S = len(slot_clusters)
    wpk = np.zeros((128, S * WSLOT), np.float32)
    bpk = np.zeros((128, S * BSLOT), np.float32)
    for s, c in enumerate(slot_clusters):
        wb, bb = s * WSLOT, s * BSLOT
        we0, we1, we2 = params["w_e0"][c], params["w_e1"][c], params["w_e2"][c]
        wd0, wd1, wd2 = params["w_d0"][c], params["w_d1"][c], params["w_d2"][c]
        for k in range(KT):
            wpk[0:P, wb + _E0 + k * 256: wb + _E0 + (k + 1) * 256] = \
                we0[P * k:P * (k + 1), :]
        for k in range(2):
            wpk[0:128, wb + _E1 + 64 * k: wb + _E1 + 64 * (k + 1)] = \
                we1[128 * k:128 * (k + 1), :]
        wpk[0:64, wb + _E2: wb + _E2 + 16] = we2
        wpk[0:16, wb + _D0: wb + _D0 + 64] = wd0
        wpk[0:64, wb + _D1: wb + _D1 + 256] = wd1
        for k in range(2):
            wpk[0:128, wb + _D2 + 784 * k: wb + _D2 + 784 * (k + 1)] = \
                wd2[128 * k:128 * (k + 1), :]

        be0, be1, be2 = params["b_e0"][c], params["b_e1"][c], params["b_e2"][c]
        bd0, bd1, bd2 = params["b_d0"][c], params["b_d1"][c], params["b_d2"][c]
        bpk[0:128, bb + 0] = be0[0:128]
        bpk[0:128, bb + 1] = be0[128:256]
        bpk[0:64, bb + 2] = be1
        bpk[0:16, bb + 3] = be2
        bpk[0:64, bb + 4] = bd0
        bpk[0:128, bb + 5] = bd1[0:128]
        bpk[0:128, bb + 6] = bd1[128:256]
        for m in range(KT):
            bpk[0:P, bb + 7 + m] = bd2[P * m:P * (m + 1)]
    return wpk, bpk


def _route(labels, mode):
    """Assign rows to (core, slot) blocks; returns config + per-slot rows."""
    counts = np.bincount(labels, minlength=K)
    configs = _CONFIGS if mode == "bf16" else _CONFIGS[1:]
    for S, R in configs:
        need = int(np.sum((counts + R - 1) // R))
        if need <= N_CORES * S:
            break
    nslots = N_CORES * S
    order = np.argsort(labels, kind="stable")
    slot_cluster = np.zeros(nslots, np.int64)
    slot_rows = [np.empty(0, np.int64)] * nslots
    si = pos = 0
    for c in range(K):
        cnt = int(counts[c])
        rows_c = order[pos:pos + cnt]
        pos += cnt
        for off in range(0, cnt, R):
            slot_cluster[si] = c
            slot_rows[si] = rows_c[off:off + R]
            si += 1
    return S, R, slot_cluster, slot_rows


def _flatten_xcore(xcore_t, R, chunks):
    """[D, S*R] feature-major slab -> chunk-flattened [P, S*R*KT]."""
    ncols = xcore_t.shape[1]
    S = ncols // R
    flat = np.empty((P, ncols * KT), np.float32)
    pos = 0
    for s in range(S):
        col = s * R
        for nch in chunks:
            blk = xcore_t[:, col:col + nch]              # [784, nch]
            blk = blk.reshape(KT, P, nch).transpose(1, 0, 2)  # [P, KT, nch]
            flat[:, pos:pos + KT * nch] = blk.reshape(P, KT * nch)
            pos += KT * nch
            col += nch
    return flat


def _unflatten_ycore(yflat, R, chunks):
    """chunk-flattened [P, S*R*KT] -> row-major [S*R, D]."""
    ncols = yflat.shape[1] // KT
    S = ncols // R
    out = np.empty((ncols, D), np.float32)
    pos = 0
    for s in range(S):
        col = s * R
        for nch in chunks:
            blk = yflat[:, pos:pos + KT * nch].reshape(P, KT, nch)
            out[col:col + nch] = blk.transpose(2, 1, 0).reshape(nch, D)
            pos += KT * nch
            col += nch
    return out


def kernel_traced(inputs, trace=False, mode=None):
    if mode is None:
        mode = MODE
    x = np.ascontiguousarray(np.asarray(inputs["x"], dtype=np.float32))
    labels = np.asarray(inputs["kmeans_label"]).astype(np.int64).ravel()
    params = {k: np.asarray(v, dtype=np.float32)
              for k, v in inputs.items() if k not in ("x", "kmeans_label")}

    S, R, slot_cluster, slot_rows = _route(labels, mode)
    chunks = _chunks(R, mode)
    nc = _get_program(S, R, mode)

    in_maps = []
    for i in range(N_CORES):
        xcore = np.zeros((S * R, D), np.float32)
        for s in range(S):
            rows = slot_rows[i * S + s]
            if len(rows):
                xcore[s * R: s * R + len(rows)] = x[rows]
        wpk, bpk = _pack_weights(params, slot_cluster[i * S:(i + 1) * S])
        xflat = _flatten_xcore(np.ascontiguousarray(xcore.T), R, chunks)
        if mode == "bf16":
            import ml_dtypes
            xflat = xflat.astype(ml_dtypes.bfloat16)
            wpk = wpk.astype(ml_dtypes.bfloat16)
        in_maps.append({"xt": xflat, "wp": wpk, "bp": bpk})

    res = run_bass_kernel_spmd(nc, in_maps, core_ids=list(range(N_CORES)),
                               trace=trace)

    out = np.zeros_like(x)
    for i in range(N_CORES):
        yraw = np.asarray(res.results[i]["yt"]).astype(np.float32)
        ytT = _unflatten_ycore(yraw, R, chunks)
        for s in range(S):
            rows = slot_rows[i * S + s]
            if len(rows):
                out[rows] = ytT[s * R: s * R + len(rows)]
    return out, res


def kernel(**inputs):
    out, _ = kernel_traced(inputs, trace=False)
    return out


# revision 11
# speedup vs baseline: 1.1435x; 1.1435x over previous
"""MoE-routed K-cluster autoencoder kernel for 8 Trainium2 NeuronCores.

Strategy
--------
Each row of x is reconstructed by the autoencoder of its kmeans cluster.
Computing all K experts densely for every row (like the reference) does
10x the needed matmul work, so we *route*:

  host:   sort rows by cluster, pack them into fixed-capacity "slots"
          (one cluster per slot; 4 slots x 1280 rows per core for uniform
          labels), pre-transpose so features lie on SBUF partitions, and
          flatten each row-chunk k-major so every DMA moves long
          contiguous runs.
  device: per slot, run the 6-layer MLP chain as feature-major matmuls
          (outT = W.T @ actsT) in fp32r (full-rate fp32) on the PE.
          Weight-stationary phase order: each weight tile is loaded once
          and streams all of the slot's row-chunks back-to-back, so the
          PE stays dense (HAM-warm) and layer-boundary waits are covered
          by the other chunks' matmuls. Bias+ReLU fuse into ScalarE
          activations that also evict PSUM->SBUF; the last layer's bias
          rides on VectorE.
  host:   scatter the per-slot outputs back to original row order.

All shapes are static; the slot capacity config adapts to the label
histogram (uniform labels always give the (4, 1280) config).
"""

import numpy as np

import concourse.tile as tile
from concourse import bacc, mybir
from concourse.bass_utils import run_bass_kernel_spmd

N_CORES = 8
B, D, H1, H2, L, K = 32768, 784, 256, 64, 16, 10
P = 112          # partition tile height for the D axis: 784 = 7 * 112
KT = D // P      # 7 k-tiles along D

# per-slot packed weight layout (column offsets in a [128, WSLOT] block)
_E0, _E1, _E2, _D0, _D1, _D2 = 0, 1792, 1920, 1936, 2000, 2256
WSLOT = 3824     # = 7*256 + 2*64 + 16 + 64 + 256 + 2*784
BSLOT = 14       # bias columns per slot: 2 + 1 + 1 + 1 + 2 + 7

# (slots_per_core, rows_per_slot)
_CONFIGS = [(4, 1152), (4, 1280), (8, 640), (16, 320), (32, 160)]

_F32 = mybir.dt.float32
_F32R = mybir.dt.float32r
_BF16 = mybir.dt.bfloat16
_RELU = mybir.ActivationFunctionType.Relu

# matmul-operand dtype mode: "f32r" (precise, fp32 bytes on the wire) or
# "bf16" (half the x/weight DMA bytes, fast weight loads, ~7e-3 rel err)
MODE = "f32r"


def _mdt_view(ap, mode):
    return ap.bitcast(_F32R) if mode == "f32r" else ap


def _chunks(R, mode="f32r"):
    """Split R rows into moving-operand chunks <=512.

    f32r: each chunk >=256 (full-rate threshold), so rebalance the tail.
    bf16: plain greedy 512s -- chunk starts must stay PSUM-bank aligned
    because matmuls write slices of one merged multi-bank psum tile."""
    out, rem = [], R
    while rem > 0:
        c = min(512, rem)
        if mode == "f32r" and c == 512 and 0 < rem - c < 256:
            c = max(256, min(512, (rem + 1) // 2))
        out.append(c)
        rem -= c
    return out


def _build_program(S, R, mode):
    mdt = _F32R if mode == "f32r" else _BF16
    idt = _F32 if mode == "f32r" else _BF16
    ncols = S * R
    nflat = ncols * KT  # x/y are stored chunk-flattened: [P, sum(KT*nch)]
    nc = bacc.Bacc("TRN2", target_bir_lowering=False, debug=False)
    xt = nc.dram_tensor("xt", [P, nflat], idt, kind="ExternalInput").ap()
    wp = nc.dram_tensor("wp", [128, S * WSLOT], idt, kind="ExternalInput").ap()
    bp = nc.dram_tensor("bp", [128, S * BSLOT], _F32, kind="ExternalInput").ap()
    yt = nc.dram_tensor("yt", [P, nflat], idt, kind="ExternalOutput").ap()

    chunks = _chunks(R, mode)
    NCH = len(chunks)

    with tile.TileContext(nc) as tc:
        with (
            tc.tile_pool(name="wpool", bufs=1) as wpool,
            tc.tile_pool(name="iopool", bufs=1) as iopool,
            tc.tile_pool(name="apool", bufs=1) as apool,
            tc.tile_pool(name="pspool", bufs=1, space="PSUM") as pspool,
        ):
            bsb = wpool.tile([128, S * BSLOT], _F32, tag="b", name="bsb", bufs=1)
            nc.sync.dma_start(out=bsb, in_=bp)

            # PE pre-warm: ~3.5us of throwaway matmuls on a zeroed tile so
            # the HAM clock-gate opens to 2.4 GHz while the first DMAs land.
            wu = wpool.tile([128, 512], mdt, tag="wu", name="wu", bufs=1)
            nc.vector.memset(wu, 0)
            wups = [pspool.tile([128, 512], _F32, tag="ps", name="wups",
                                bufs=8) for _ in range(4)]
            for i in range(16):
                nc.tensor.matmul(wups[i % 4], wu[:, 0:128], wu,
                                 start=True, stop=True)

            def bias(lo, col):
                return bsb[0:lo, col:col + 1]

            def ps_tile(parts, nch):
                return pspool.tile([parts, nch], _F32, tag="ps", name="ps",
                                   bufs=8)

            drain_i = [0]

            def drain_relu(out, ps, bias_ap):
                """bias+ReLU PSUM->SBUF eviction, alternating ACT/DVE."""
                drain_i[0] += 1
                if drain_i[0] % 2:
                    nc.scalar.activation(out, ps, _RELU, bias=bias_ap)
                else:
                    nc.vector.tensor_scalar(out, ps, bias_ap, 0.0,
                                            mybir.AluOpType.add,
                                            mybir.AluOpType.max)

            def drain_bias(out, ps, bias_ap):
                """bias-only PSUM->SBUF eviction, alternating ACT/DVE."""
                drain_i[0] += 1
                if drain_i[0] % 2:
                    nc.scalar.add(out, ps, bias_ap)
                else:
                    nc.vector.tensor_scalar_add(out, ps, bias_ap)

            for s in range(S):
                w = wpool.tile([128, WSLOT], mdt, tag="w", name="w", bufs=2)
                nc.sync.dma_start(
                    out=w, in_=_mdt_view(wp[:, s * WSLOT:(s + 1) * WSLOT], mode))
                bb = s * BSLOT

                # chunk-flattened column offsets for this slot
                offs = []
                cum = s * R * KT
                for nch in chunks:
                    offs.append(cum)
                    cum += nch * KT

                xq = []
                for ci, nch in enumerate(chunks):
                    t = iopool.tile([128, KT, nch], mdt, tag="xq", name="xq",
                                    bufs=NCH + (3 if mode == "bf16" else 2))
                    nc.sync.dma_start(
                        out=t[0:P],
                        in_=_mdt_view(xt[:, offs[ci]:offs[ci] + KT * nch]
                        .rearrange("p (k n) -> p k n", k=KT), mode))
                    xq.append(t)

                # encoder 0: [784 -> 256]; weight-stationary over chunks
                h1 = [[None] * NCH, [None] * NCH]
                for m in range(2):
                    ps = [None] * NCH
                    for k in range(KT):
                        wk = w[0:P, _E0 + k * 256 + 128 * m:
                               _E0 + k * 256 + 128 * m + 128]
                        for ci, nch in enumerate(chunks):
                            if k == 0:
                                ps[ci] = ps_tile(128, nch)
                            nc.tensor.matmul(ps[ci], wk, xq[ci][0:P, k, :],
                                             start=(k == 0), stop=(k == KT - 1))
                    for ci, nch in enumerate(chunks):
                        t = apool.tile([128, nch], mdt, tag="h1", name="h1",
                                       bufs=7)
                        drain_relu(t, ps[ci], bias(128, bb + m))
                        h1[m][ci] = t

                # encoder 1: [256 -> 64]
                ps = [None] * NCH
                for k in range(2):
                    wk = w[0:128, _E1 + 64 * k:_E1 + 64 * k + 64]
                    for ci, nch in enumerate(chunks):
                        if k == 0:
                            ps[ci] = ps_tile(64, nch)
                        nc.tensor.matmul(ps[ci], wk, h1[k][ci],
                                         start=(k == 0), stop=(k == 1))
                h2 = []
                for ci, nch in enumerate(chunks):
                    t = apool.tile([64, nch], mdt, tag="h2", name="h2",
                                   bufs=4)
                    drain_relu(t, ps[ci], bias(64, bb + 2))
                    h2.append(t)

                # encoder 2: [64 -> 16]
                ps = [None] * NCH
                wk = w[0:64, _E2:_E2 + 16]
                for ci, nch in enumerate(chunks):
                    ps[ci] = ps_tile(16, nch)
                    nc.tensor.matmul(ps[ci], wk, h2[ci], start=True, stop=True)
                z = []
                for ci, nch in enumerate(chunks):
                    t = apool.tile([16, nch], mdt, tag="z", name="z",
                                   bufs=4)
                    drain_relu(t, ps[ci], bias(16, bb + 3))
                    z.append(t)

                # decoder 0: [16 -> 64]
                ps = [None] * NCH
                wk = w[0:16, _D0:_D0 + 64]
                for ci, nch in enumerate(chunks):
                    ps[ci] = ps_tile(64, nch)
                    nc.tensor.matmul(ps[ci], wk, z[ci], start=True, stop=True)
                a1 = []
                for ci, nch in enumerate(chunks):
                    t = apool.tile([64, nch], mdt, tag="a1", name="a1",
                                   bufs=4)
                    drain_relu(t, ps[ci], bias(64, bb + 4))
                    a1.append(t)

                # decoder 1: [64 -> 256]
                a2 = [[None] * NCH, [None] * NCH]
                for m in range(2):
                    wk = w[0:64, _D1 + 128 * m:_D1 + 128 * m + 128]
                    ps = [None] * NCH
                    for ci, nch in enumerate(chunks):
                        ps[ci] = ps_tile(128, nch)
                        nc.tensor.matmul(ps[ci], wk, a1[ci], start=True, stop=True)
                    for ci, nch in enumerate(chunks):
                        t = apool.tile([128, nch], mdt, tag="a2", name="a2",
                                       bufs=7)
                        nc.scalar.activation(t, ps[ci], _RELU,
                                             bias=bias(128, bb + 5 + m))
                        a2[m][ci] = t

                # decoder 2: [256 -> 784], bias only (VectorE)
                yq = []
                for ci, nch in enumerate(chunks):
                    yq.append(iopool.tile([128, KT, nch], idt, tag="yq",
                                          name="yq", bufs=NCH + 1))
                for mm in range(KT):
                    ps = [None] * NCH
                    for k in range(2):
                        wk = w[0:128, _D2 + 784 * k + 112 * mm:
                               _D2 + 784 * k + 112 * mm + 112]
                        for ci, nch in enumerate(chunks):
                            if k == 0:
                                ps[ci] = ps_tile(112, nch)
                            nc.tensor.matmul(ps[ci], wk, a2[k][ci],
                                             start=(k == 0), stop=(k == 1))
                    for ci, nch in enumerate(chunks):
                        drain_bias(yq[ci][0:P, mm, :], ps[ci],
                                   bias(112, bb + 7 + mm))
                for ci, nch in enumerate(chunks):
                    nc.sync.dma_start(
                        out=yt[:, offs[ci]:offs[ci] + KT * nch]
                        .rearrange("p (k n) -> p k n", k=KT),
                        in_=yq[ci][0:P])
    nc.compile()
    return nc




def _build_program_v3(S, R):
    """bf16 path: slot-level x/y tiles (one DMA each), merged multi-bank
    PSUM accumulator per weight group (one drain op instead of one per
    chunk), drains balanced across ScalarE/VectorE, bf16 output writes."""
    ncols = S * R
    nflat = ncols * KT
    nc = bacc.Bacc("TRN2", target_bir_lowering=False, debug=False)
    xt = nc.dram_tensor("xt", [P, nflat], _BF16, kind="ExternalInput").ap()
    wp = nc.dram_tensor("wp", [128, S * WSLOT], _BF16, kind="ExternalInput").ap()
    bp = nc.dram_tensor("bp", [128, S * BSLOT], _F32, kind="ExternalInput").ap()
    yt = nc.dram_tensor("yt", [P, nflat], _BF16, kind="ExternalOutput").ap()

    chunks = _chunks(R, "bf16")
    slices = []
    off = 0
    for nch in chunks:
        slices.append((off, nch))
        off += nch

    with tile.TileContext(nc) as tc:
        with (
            tc.tile_pool(name="wpool", bufs=1) as wpool,
            tc.tile_pool(name="iopool", bufs=1) as iopool,
            tc.tile_pool(name="apool", bufs=1) as apool,
            tc.tile_pool(name="pspool", bufs=1, space="PSUM") as pspool,
        ):
            bsb = wpool.tile([128, S * BSLOT], _F32, tag="b", name="bsb", bufs=1)
            nc.sync.dma_start(out=bsb, in_=bp)

            # PE pre-warm: ~3.5us of throwaway matmuls on a zeroed tile so
            # the HAM clock-gate opens to 2.4 GHz while the first DMAs land.
            wu = wpool.tile([128, 512], mdt, tag="wu", name="wu", bufs=1)
            nc.vector.memset(wu, 0)
            wups = [pspool.tile([128, 512], _F32, tag="ps", name="wups",
                                bufs=8) for _ in range(4)]
            for i in range(16):
                nc.tensor.matmul(wups[i % 4], wu[:, 0:128], wu,
                                 start=True, stop=True)

            def bias(lo, col):
                return bsb[0:lo, col:col + 1]

            def mm_group(ps_parts, w_tiles, rhs_of):
                """One merged accumulator: ps[:, off:off+nch] accumulates
                w_tiles[k].T @ rhs_of(k)[:, off:off+nch] over k."""
                ps = pspool.tile([ps_parts, R], _F32, tag="ps", name="ps",
                                 bufs=2)
                nk = len(w_tiles)
                for k in range(nk):
                    for off, nch in slices:
                        nc.tensor.matmul(ps[:, off:off + nch], w_tiles[k],
                                         rhs_of(k)[:, off:off + nch],
                                         start=(k == 0), stop=(k == nk - 1))
                return ps

            for s in range(S):
                w = wpool.tile([128, WSLOT], _BF16, tag="w", name="w", bufs=2)
                nc.sync.dma_start(out=w, in_=wp[:, s * WSLOT:(s + 1) * WSLOT])
                bb = s * BSLOT

                xq = iopool.tile([128, KT, R], _BF16, tag="xq", name="xq",
                                 bufs=3)
                nc.sync.dma_start(
                    out=xq[0:P],
                    in_=xt[:, s * R * KT:(s + 1) * R * KT]
                    .rearrange("p (k n) -> p k n", k=KT))

                # encoder 0: [784 -> 256]
                h1 = []
                for m in range(2):
                    ps = mm_group(
                        128,
                        [w[0:P, _E0 + k * 256 + 128 * m:
                           _E0 + k * 256 + 128 * m + 128] for k in range(KT)],
                        lambda k: xq[0:P, k, :])
                    t = apool.tile([128, R], _BF16, tag="h1", name="h1", bufs=3)
                    nc.scalar.activation(t, ps, _RELU, bias=bias(128, bb + m))
                    h1.append(t)

                # encoder 1: [256 -> 64]
                ps = mm_group(64, [w[0:128, _E1 + 64 * k:_E1 + 64 * k + 64]
                                   for k in range(2)], lambda k: h1[k])
                h2 = apool.tile([64, R], _BF16, tag="h2", name="h2", bufs=2)
                nc.scalar.activation(h2, ps, _RELU, bias=bias(64, bb + 2))

                # encoder 2: [64 -> 16]
                ps = mm_group(16, [w[0:64, _E2:_E2 + 16]], lambda k: h2)
                z = apool.tile([16, R], _BF16, tag="z", name="z", bufs=2)
                nc.scalar.activation(z, ps, _RELU, bias=bias(16, bb + 3))

                # decoder 0: [16 -> 64]
                ps = mm_group(64, [w[0:16, _D0:_D0 + 64]], lambda k: z)
                a1 = apool.tile([64, R], _BF16, tag="a1", name="a1", bufs=2)
                nc.scalar.activation(a1, ps, _RELU, bias=bias(64, bb + 4))

                # decoder 1: [64 -> 256]
                a2 = []
                for m in range(2):
                    ps = mm_group(128, [w[0:64, _D1 + 128 * m:
                                          _D1 + 128 * m + 128]], lambda k: a1)
                    t = apool.tile([128, R], _BF16, tag="a2", name="a2", bufs=3)
                    nc.scalar.activation(t, ps, _RELU,
                                         bias=bias(128, bb + 5 + m))
                    a2.append(t)

                # decoder 2: [256 -> 784], bias only; balance ACT vs DVE
                yq = iopool.tile([128, KT, R], _BF16, tag="yq", name="yq",
                                 bufs=2)
                for mm in range(KT):
                    ps = mm_group(
                        112,
                        [w[0:128, _D2 + 784 * k + 112 * mm:
                           _D2 + 784 * k + 112 * mm + 112] for k in range(2)],
                        lambda k: a2[k])
                    if mm < 2:
                        nc.scalar.add(yq[0:P, mm, :], ps, bias(112, bb + 7 + mm))
                    else:
                        nc.vector.tensor_scalar_add(
                            yq[0:P, mm, :], ps, bias(112, bb + 7 + mm))
                nc.sync.dma_start(
                    out=yt[:, s * R * KT:(s + 1) * R * KT]
                    .rearrange("p (k n) -> p k n", k=KT),
                    in_=yq[0:P])
    nc.compile()
    return nc


_programs = {}


def _get_program(S, R, mode):
    if (S, R, mode) not in _programs:
        _programs[(S, R, mode)] = _build_program(S, R, mode)
    return _programs[(S, R, mode)]


def _pack_weights(params, slot_clusters):
    S = len(slot_clusters)
    wpk = np.zeros((128, S * WSLOT), np.float32)
    bpk = np.zeros((128, S * BSLOT), np.float32)
    for s, c in enumerate(slot_clusters):
        wb, bb = s * WSLOT, s * BSLOT
        we0, we1, we2 = params["w_e0"][c], params["w_e1"][c], params["w_e2"][c]
        wd0, wd1, wd2 = params["w_d0"][c], params["w_d1"][c], params["w_d2"][c]
        for k in range(KT):
            wpk[0:P, wb + _E0 + k * 256: wb + _E0 + (k + 1) * 256] = \
                we0[P * k:P * (k + 1), :]
        for k in range(2):
            wpk[0:128, wb + _E1 + 64 * k: wb + _E1 + 64 * (k + 1)] = \
                we1[128 * k:128 * (k + 1), :]
        wpk[0:64, wb + _E2: wb + _E2 + 16] = we2
        wpk[0:16, wb + _D0: wb + _D0 + 64] = wd0
        wpk[0:64, wb + _D1: wb + _D1 + 256] = wd1
        for k in range(2):
            wpk[0:128, wb + _D2 + 784 * k: wb + _D2 + 784 * (k + 1)] = \
                wd2[128 * k:128 * (k + 1), :]

        be0, be1, be2 = params["b_e0"][c], params["b_e1"][c], params["b_e2"][c]
        bd0, bd1, bd2 = params["b_d0"][c], params["b_d1"][c], params["b_d2"][c]
        bpk[0:128, bb + 0] = be0[0:128]
        bpk[0:128, bb + 1] = be0[128:256]
        bpk[0:64, bb + 2] = be1
        bpk[0:16, bb + 3] = be2
        bpk[0:64, bb + 4] = bd0
        bpk[0:128, bb + 5] = bd1[0:128]
        bpk[0:128, bb + 6] = bd1[128:256]
        for m in range(KT):
            bpk[0:P, bb + 7 + m] = bd2[P * m:P * (m + 1)]
    return wpk, bpk


def _route(labels, mode):
    """Assign rows to (core, slot) blocks; returns config + per-slot rows."""
    counts = np.bincount(labels, minlength=K)
    configs = _CONFIGS if mode == "bf16" else _CONFIGS[1:]
    for S, R in configs:
        need = int(np.sum((counts + R - 1) // R))
        if need <= N_CORES * S:
            break
    nslots = N_CORES * S
    order = np.argsort(labels, kind="stable")
    slot_cluster = np.zeros(nslots, np.int64)
    slot_rows = [np.empty(0, np.int64)] * nslots
    si = pos = 0
    for c in range(K):
        cnt = int(counts[c])
        rows_c = order[pos:pos + cnt]
        pos += cnt
        for off in range(0, cnt, R):
            slot_cluster[si] = c
            slot_rows[si] = rows_c[off:off + R]
            si += 1
    return S, R, slot_cluster, slot_rows


def _flatten_xcore(xcore_t, R, chunks):
    """[D, S*R] feature-major slab -> chunk-flattened [P, S*R*KT]."""
    ncols = xcore_t.shape[1]
    S = ncols // R
    flat = np.empty((P, ncols * KT), np.float32)
    pos = 0
    for s in range(S):
        col = s * R
        for nch in chunks:
            blk = xcore_t[:, col:col + nch]              # [784, nch]
            blk = blk.reshape(KT, P, nch).transpose(1, 0, 2)  # [P, KT, nch]
            flat[:, pos:pos + KT * nch] = blk.reshape(P, KT * nch)
            pos += KT * nch
            col += nch
    return flat


def _unflatten_ycore(yflat, R, chunks):
    """chunk-flattened [P, S*R*KT] -> row-major [S*R, D]."""
    ncols = yflat.shape[1] // KT
    S = ncols // R
    out = np.empty((ncols, D), np.float32)
    pos = 0
    for s in range(S):
        col = s * R
        for nch in chunks:
            blk = yflat[:, pos:pos + KT * nch].reshape(P, KT, nch)
            out[col:col + nch] = blk.transpose(2, 1, 0).reshape(nch, D)
            pos += KT * nch
            col += nch
    return out


def kernel_traced(inputs, trace=False, mode=None):
    if mode is None:
        mode = MODE
    x = np.ascontiguousarray(np.asarray(inputs["x"], dtype=np.float32))
    labels = np.asarray(inputs["kmeans_label"]).astype(np.int64).ravel()
    params = {k: np.asarray(v, dtype=np.float32)
              for k, v in inputs.items() if k not in ("x", "kmeans_label")}

    S, R, slot_cluster, slot_rows = _route(labels, mode)
    chunks = _chunks(R, mode)
    nc = _get_program(S, R, mode)

    in_maps = []
    for i in range(N_CORES):
        xcore = np.zeros((S * R, D), np.float32)
        for s in range(S):
            rows = slot_rows[i * S + s]
            if len(rows):
                xcore[s * R: s * R + len(rows)] = x[rows]
        wpk, bpk = _pack_weights(params, slot_cluster[i * S:(i + 1) * S])
        xflat = _flatten_xcore(np.ascontiguousarray(xcore.T), R, chunks)
        if mode == "bf16":
            import ml_dtypes
            xflat = xflat.astype(ml_dtypes.bfloat16)
            wpk = wpk.astype(ml_dtypes.bfloat16)
        in_maps.append({"xt": xflat, "wp": wpk, "bp": bpk})

    res = run_bass_kernel_spmd(nc, in_maps, core_ids=list(range(N_CORES)),
                               trace=trace)

    out = np.zeros_like(x)
    for i in range(N_CORES):
        yraw = np.asarray(res.results[i]["yt"]).astype(np.float32)
        ytT = _unflatten_ycore(yraw, R, chunks)
        for s in range(S):
            rows = slot_rows[i * S + s]
            if len(rows):
                out[rows] = ytT[s * R: s * R + len(rows)]
    return out, res


def kernel(**inputs):
    out, _ = kernel_traced(inputs, trace=False)
    return out


# revision 14
# speedup vs baseline: 1.2165x; 1.0639x over previous
"""MoE-routed K-cluster autoencoder kernel for 8 Trainium2 NeuronCores.

Strategy
--------
Each row of x is reconstructed by the autoencoder of its kmeans cluster.
Computing all K experts densely for every row (like the reference) does
10x the needed matmul work, so we *route*:

  host:   sort rows by cluster, pack them into fixed-capacity "slots"
          (one cluster per slot; 4 slots x 1280 rows per core for uniform
          labels), pre-transpose so features lie on SBUF partitions, and
          flatten each row-chunk k-major so every DMA moves long
          contiguous runs.
  device: per slot, run the 6-layer MLP chain as feature-major matmuls
          (outT = W.T @ actsT) in fp32r (full-rate fp32) on the PE.
          Weight-stationary phase order: each weight tile is loaded once
          and streams all of the slot's row-chunks back-to-back, so the
          PE stays dense (HAM-warm) and layer-boundary waits are covered
          by the other chunks' matmuls. Bias+ReLU fuse into ScalarE
          activations that also evict PSUM->SBUF; the last layer's bias
          rides on VectorE.
  host:   scatter the per-slot outputs back to original row order.

All shapes are static; the slot capacity config adapts to the label
histogram (uniform labels always give the (4, 1280) config).
"""

import numpy as np

import concourse.tile as tile
from concourse import bacc, mybir
from concourse.bass_utils import run_bass_kernel_spmd

N_CORES = 8
B, D, H1, H2, L, K = 32768, 784, 256, 64, 16, 10
P = 112          # partition tile height for the D axis: 784 = 7 * 112
KT = D // P      # 7 k-tiles along D

# per-slot packed weight layout (column offsets in a [128, WSLOT] block)
_E0, _E1, _E2, _D0, _D1, _D2 = 0, 1792, 1920, 1936, 2000, 2256
WSLOT = 3824     # = 7*256 + 2*64 + 16 + 64 + 256 + 2*784
BSLOT = 14       # bias columns per slot: 2 + 1 + 1 + 1 + 2 + 7

# (slots_per_core, rows_per_slot)
_CONFIGS = [(4, 1152), (4, 1280), (8, 640), (16, 320), (32, 160)]

_F32 = mybir.dt.float32
_F32R = mybir.dt.float32r
_BF16 = mybir.dt.bfloat16
_RELU = mybir.ActivationFunctionType.Relu

# matmul-operand dtype mode: "f32r" (precise, fp32 bytes on the wire) or
# "bf16" (half the x/weight DMA bytes, fast weight loads, ~7e-3 rel err)
MODE = "f32r"


def _mdt_view(ap, mode):
    return ap.bitcast(_F32R) if mode == "f32r" else ap


def _chunks(R, mode="f32r"):
    """Split R rows into moving-operand chunks <=512.

    f32r: each chunk >=256 (full-rate threshold), so rebalance the tail.
    bf16: plain greedy 512s -- chunk starts must stay PSUM-bank aligned
    because matmuls write slices of one merged multi-bank psum tile."""
    out, rem = [], R
    while rem > 0:
        c = min(512, rem)
        if mode == "f32r" and c == 512 and 0 < rem - c < 256:
            c = max(256, min(512, (rem + 1) // 2))
        out.append(c)
        rem -= c
    return out


def _build_program(S, R, mode):
    mdt = _F32R if mode == "f32r" else _BF16
    idt = _F32 if mode == "f32r" else _BF16
    pipelined = mode == "bf16"
    ncols = S * R
    nflat = ncols * KT  # x/y are stored chunk-flattened: [P, sum(KT*nch)]
    nc = bacc.Bacc("TRN2", target_bir_lowering=False, debug=False)
    xt = nc.dram_tensor("xt", [P, nflat], idt, kind="ExternalInput").ap()
    wp = nc.dram_tensor("wp", [128, S * WSLOT], idt, kind="ExternalInput").ap()
    bp = nc.dram_tensor("bp", [128, S * BSLOT], _F32, kind="ExternalInput").ap()
    yt = nc.dram_tensor("yt", [P, nflat], idt, kind="ExternalOutput").ap()

    chunks = _chunks(R, mode)
    NCH = len(chunks)
    XQ_BUFS = 3 * NCH if pipelined else NCH + 2
    W_BUFS = 3 if pipelined else 2
    H1_BUFS = 8 if pipelined else 6
    SM_BUFS = 4 if pipelined else 3

    with tile.TileContext(nc) as tc:
        with (
            tc.tile_pool(name="wpool", bufs=1) as wpool,
            tc.tile_pool(name="iopool", bufs=1) as iopool,
            tc.tile_pool(name="apool", bufs=1) as apool,
            tc.tile_pool(name="pspool", bufs=1, space="PSUM") as pspool,
        ):
            bsb = wpool.tile([128, S * BSLOT], _F32, tag="b", name="bsb", bufs=1)
            nc.sync.dma_start(out=bsb, in_=bp)

            # PE pre-warm: ~3.5us of throwaway matmuls on a zeroed tile so
            # the HAM clock-gate opens to 2.4 GHz while the first DMAs land.
            wu = wpool.tile([128, 512], _BF16, tag="wu", name="wu", bufs=1)
            nc.vector.memset(wu, 0)
            wups = [pspool.tile([128, 512], _F32, tag="ps", name="wups",
                                bufs=8) for _ in range(4)]
            for i in range(16):
                nc.tensor.matmul(wups[i % 4], wu[:, 0:128], wu,
                                 start=True, stop=True)

            def bias(lo, col):
                return bsb[0:lo, col:col + 1]

            def ps_tile(parts, nch):
                return pspool.tile([parts, nch], _F32, tag="ps", name="ps",
                                   bufs=8)

            drain_i = [0]

            def drain_relu(out, ps, bias_ap):
                """bias+ReLU PSUM->SBUF eviction, alternating ACT/DVE."""
                drain_i[0] += 1
                if drain_i[0] % 2:
                    nc.scalar.activation(out, ps, _RELU, bias=bias_ap)
                else:
                    nc.vector.tensor_scalar(out, ps, bias_ap, 0.0,
                                            mybir.AluOpType.add,
                                            mybir.AluOpType.max)

            def drain_bias(out, ps, bias_ap):
                """bias-only PSUM->SBUF eviction, alternating ACT/DVE."""
                drain_i[0] += 1
                if drain_i[0] % 2:
                    nc.scalar.add(out, ps, bias_ap)
                else:
                    nc.vector.tensor_scalar_add(out, ps, bias_ap)

            res = {}

            def ensure_slot(s):
                """Allocate slot s's weight/x tiles and issue their DMAs."""
                if s in res or s >= S:
                    return
                w = wpool.tile([128, WSLOT], mdt, tag="w", name="w",
                               bufs=W_BUFS)
                nc.sync.dma_start(
                    out=w,
                    in_=_mdt_view(wp[:, s * WSLOT:(s + 1) * WSLOT], mode))
                offs = []
                cum = s * R * KT
                for nch in chunks:
                    offs.append(cum)
                    cum += nch * KT
                xq = []
                for ci, nch in enumerate(chunks):
                    t = iopool.tile([128, KT, nch], mdt, tag="xq", name="xq",
                                    bufs=XQ_BUFS)
                    nc.sync.dma_start(
                        out=t[0:P],
                        in_=_mdt_view(
                            xt[:, offs[ci]:offs[ci] + KT * nch]
                            .rearrange("p (k n) -> p k n", k=KT), mode))
                    xq.append(t)
                res[s] = {"w": w, "xq": xq, "offs": offs, "bb": s * BSLOT,
                          "h1": [[None] * NCH, [None] * NCH],
                          "e0ps": [None, None]}

            def e0_group(s, m, k):
                """One weight-stationary e0 group: [784->256] m-half, k-tile."""
                r = res[s]
                if k == 0:
                    r["e0ps"][m] = [ps_tile(128, nch) for nch in chunks]
                wk = r["w"][0:P, _E0 + k * 256 + 128 * m:
                            _E0 + k * 256 + 128 * m + 128]
                for ci, nch in enumerate(chunks):
                    nc.tensor.matmul(r["e0ps"][m][ci], wk,
                                     r["xq"][ci][0:P, k, :],
                                     start=(k == 0), stop=(k == KT - 1))
                if k == KT - 1:
                    for ci, nch in enumerate(chunks):
                        t = apool.tile([128, nch], mdt, tag="h1", name="h1",
                                       bufs=H1_BUFS)
                        drain_relu(t, r["e0ps"][m][ci], bias(128, r["bb"] + m))
                        r["h1"][m][ci] = t
                    r["e0ps"][m] = None

            E0_ORDER = [(m, k) for m in range(2) for k in range(KT)]

            if pipelined:
                ensure_slot(0)
                ensure_slot(1)
                for m, k in E0_ORDER:
                    e0_group(0, m, k)

            for s in range(S):
                if pipelined:
                    ensure_slot(s + 2)
                    filler = iter(E0_ORDER) if s + 1 < S else iter([])
                else:
                    ensure_slot(s)
                    for m, k in E0_ORDER:
                        e0_group(s, m, k)
                    filler = iter([])

                def fill(n):
                    for _ in range(n):
                        mk = next(filler, None)
                        if mk is not None:
                            e0_group(s + 1, *mk)

                r = res[s]
                w, bb, offs, h1 = r["w"], r["bb"], r["offs"], r["h1"]

                # encoder 1: [256 -> 64]
                ps = [None] * NCH
                for k in range(2):
                    wk = w[0:128, _E1 + 64 * k:_E1 + 64 * k + 64]
                    for ci, nch in enumerate(chunks):
                        if k == 0:
                            ps[ci] = ps_tile(64, nch)
                        nc.tensor.matmul(ps[ci], wk, h1[k][ci],
                                         start=(k == 0), stop=(k == 1))
                h2 = []
                for ci, nch in enumerate(chunks):
                    t = apool.tile([64, nch], mdt, tag="h2", name="h2", bufs=SM_BUFS)
                    drain_relu(t, ps[ci], bias(64, bb + 2))
                    h2.append(t)
                fill(2)

                # encoder 2: [64 -> 16]
                ps = [None] * NCH
                wk = w[0:64, _E2:_E2 + 16]
                for ci, nch in enumerate(chunks):
                    ps[ci] = ps_tile(16, nch)
                    nc.tensor.matmul(ps[ci], wk, h2[ci], start=True, stop=True)
                z = []
                for ci, nch in enumerate(chunks):
                    t = apool.tile([16, nch], mdt, tag="z", name="z", bufs=SM_BUFS)
                    drain_relu(t, ps[ci], bias(16, bb + 3))
                    z.append(t)
                fill(2)

                # decoder 0: [16 -> 64]
                ps = [None] * NCH
                wk = w[0:16, _D0:_D0 + 64]
                for ci, nch in enumerate(chunks):
                    ps[ci] = ps_tile(64, nch)
                    nc.tensor.matmul(ps[ci], wk, z[ci], start=True, stop=True)
                a1 = []
                for ci, nch in enumerate(chunks):
                    t = apool.tile([64, nch], mdt, tag="a1", name="a1", bufs=SM_BUFS)
                    drain_relu(t, ps[ci], bias(64, bb + 4))
                    a1.append(t)
                fill(2)

                # decoder 1: [64 -> 256]
                a2 = [[None] * NCH, [None] * NCH]
                for m in range(2):
                    wk = w[0:64, _D1 + 128 * m:_D1 + 128 * m + 128]
                    ps = [None] * NCH
                    for ci, nch in enumerate(chunks):
                        ps[ci] = ps_tile(128, nch)
                        nc.tensor.matmul(ps[ci], wk, a1[ci],
                                         start=True, stop=True)
                    for ci, nch in enumerate(chunks):
                        t = apool.tile([128, nch], mdt, tag="a2", name="a2",
                                       bufs=7)
                        drain_relu(t, ps[ci], bias(128, bb + 5 + m))
                        a2[m][ci] = t
                    fill(2)

                # decoder 2: [256 -> 784], bias only
                yq = []
                for ci, nch in enumerate(chunks):
                    yq.append(iopool.tile([128, KT, nch], idt, tag="yq",
                                          name="yq", bufs=NCH + 1))
                for mm in range(KT):
                    ps = [None] * NCH
                    for k in range(2):
                        wk = w[0:128, _D2 + 784 * k + 112 * mm:
                               _D2 + 784 * k + 112 * mm + 112]
                        for ci, nch in enumerate(chunks):
                            if k == 0:
                                ps[ci] = ps_tile(112, nch)
                            nc.tensor.matmul(ps[ci], wk, a2[k][ci],
                                             start=(k == 0), stop=(k == 1))
                    for ci, nch in enumerate(chunks):
                        drain_bias(yq[ci][0:P, mm, :], ps[ci],
                                   bias(112, bb + 7 + mm))
                    if mm < 4:
                        fill(1)
                fill(14)
                for ci, nch in enumerate(chunks):
                    nc.sync.dma_start(
                        out=yt[:, offs[ci]:offs[ci] + KT * nch]
                        .rearrange("p (k n) -> p k n", k=KT),
                        in_=yq[ci][0:P])
                del res[s]
    nc.compile()
    return nc


def _build_program_v3(S, R):
    """bf16 path: slot-level x/y tiles (one DMA each), merged multi-bank
    PSUM accumulator per weight group (one drain op instead of one per
    chunk), drains balanced across ScalarE/VectorE, bf16 output writes."""
    ncols = S * R
    nflat = ncols * KT
    nc = bacc.Bacc("TRN2", target_bir_lowering=False, debug=False)
    xt = nc.dram_tensor("xt", [P, nflat], _BF16, kind="ExternalInput").ap()
    wp = nc.dram_tensor("wp", [128, S * WSLOT], _BF16, kind="ExternalInput").ap()
    bp = nc.dram_tensor("bp", [128, S * BSLOT], _F32, kind="ExternalInput").ap()
    yt = nc.dram_tensor("yt", [P, nflat], _BF16, kind="ExternalOutput").ap()

    chunks = _chunks(R, "bf16")
    slices = []
    off = 0
    for nch in chunks:
        slices.append((off, nch))
        off += nch

    with tile.TileContext(nc) as tc:
        with (
            tc.tile_pool(name="wpool", bufs=1) as wpool,
            tc.tile_pool(name="iopool", bufs=1) as iopool,
            tc.tile_pool(name="apool", bufs=1) as apool,
            tc.tile_pool(name="pspool", bufs=1, space="PSUM") as pspool,
        ):
            bsb = wpool.tile([128, S * BSLOT], _F32, tag="b", name="bsb", bufs=1)
            nc.sync.dma_start(out=bsb, in_=bp)

            # PE pre-warm: ~3.5us of throwaway matmuls on a zeroed tile so
            # the HAM clock-gate opens to 2.4 GHz while the first DMAs land.
            wu = wpool.tile([128, 512], _BF16, tag="wu", name="wu", bufs=1)
            nc.vector.memset(wu, 0)
            wups = [pspool.tile([128, 512], _F32, tag="ps", name="wups",
                                bufs=8) for _ in range(4)]
            for i in range(16):
                nc.tensor.matmul(wups[i % 4], wu[:, 0:128], wu,
                                 start=True, stop=True)

            def bias(lo, col):
                return bsb[0:lo, col:col + 1]

            def mm_group(ps_parts, w_tiles, rhs_of):
                """One merged accumulator: ps[:, off:off+nch] accumulates
                w_tiles[k].T @ rhs_of(k)[:, off:off+nch] over k."""
                ps = pspool.tile([ps_parts, R], _F32, tag="ps", name="ps",
                                 bufs=2)
                nk = len(w_tiles)
                for k in range(nk):
                    for off, nch in slices:
                        nc.tensor.matmul(ps[:, off:off + nch], w_tiles[k],
                                         rhs_of(k)[:, off:off + nch],
                                         start=(k == 0), stop=(k == nk - 1))
                return ps

            for s in range(S):
                w = wpool.tile([128, WSLOT], _BF16, tag="w", name="w", bufs=2)
                nc.sync.dma_start(out=w, in_=wp[:, s * WSLOT:(s + 1) * WSLOT])
                bb = s * BSLOT

                xq = iopool.tile([128, KT, R], _BF16, tag="xq", name="xq",
                                 bufs=3)
                nc.sync.dma_start(
                    out=xq[0:P],
                    in_=xt[:, s * R * KT:(s + 1) * R * KT]
                    .rearrange("p (k n) -> p k n", k=KT))

                # encoder 0: [784 -> 256]
                h1 = []
                for m in range(2):
                    ps = mm_group(
                        128,
                        [w[0:P, _E0 + k * 256 + 128 * m:
                           _E0 + k * 256 + 128 * m + 128] for k in range(KT)],
                        lambda k: xq[0:P, k, :])
                    t = apool.tile([128, R], _BF16, tag="h1", name="h1", bufs=3)
                    nc.scalar.activation(t, ps, _RELU, bias=bias(128, bb + m))
                    h1.append(t)

                # encoder 1: [256 -> 64]
                ps = mm_group(64, [w[0:128, _E1 + 64 * k:_E1 + 64 * k + 64]
                                   for k in range(2)], lambda k: h1[k])
                h2 = apool.tile([64, R], _BF16, tag="h2", name="h2", bufs=2)
                nc.scalar.activation(h2, ps, _RELU, bias=bias(64, bb + 2))

                # encoder 2: [64 -> 16]
                ps = mm_group(16, [w[0:64, _E2:_E2 + 16]], lambda k: h2)
                z = apool.tile([16, R], _BF16, tag="z", name="z", bufs=2)
                nc.scalar.activation(z, ps, _RELU, bias=bias(16, bb + 3))

                # decoder 0: [16 -> 64]
                ps = mm_group(64, [w[0:16, _D0:_D0 + 64]], lambda k: z)
                a1 = apool.tile([64, R], _BF16, tag="a1", name="a1", bufs=2)
                nc.scalar.activation(a1, ps, _RELU, bias=bias(64, bb + 4))

                # decoder 1: [64 -> 256]
                a2 = []
                for m in range(2):
                    ps = mm_group(128, [w[0:64, _D1 + 128 * m:
                                          _D1 + 128 * m + 128]], lambda k: a1)
                    t = apool.tile([128, R], _BF16, tag="a2", name="a2", bufs=3)
                    nc.scalar.activation(t, ps, _RELU,
                                         bias=bias(128, bb + 5 + m))
                    a2.append(t)

                # decoder 2: [256 -> 784], bias only; balance ACT vs DVE
                yq = iopool.tile([128, KT, R], _BF16, tag="yq", name="yq",
                                 bufs=2)
                for mm in range(KT):
                    ps = mm_group(
                        112,
                        [w[0:128, _D2 + 784 * k + 112 * mm:
                           _D2 + 784 * k + 112 * mm + 112] for k in range(2)],
                        lambda k: a2[k])
                    if mm < 2:
                        nc.scalar.add(yq[0:P, mm, :], ps, bias(112, bb + 7 + mm))
                    else:
                        nc.vector.tensor_scalar_add(
                            yq[0:P, mm, :], ps, bias(112, bb + 7 + mm))
                nc.sync.dma_start(
                    out=yt[:, s * R * KT:(s + 1) * R * KT]
                    .rearrange("p (k n) -> p k n", k=KT),
                    in_=yq[0:P])
    nc.compile()
    return nc


_programs = {}


def _get_program(S, R, mode):
    if (S, R, mode) not in _programs:
        _programs[(S, R, mode)] = _build_program(S, R, mode)
    return _programs[(S, R, mode)]


def _pack_weights(params, slot_clusters):
    S = len(slot_clusters)
    wpk = np.zeros((128, S * WSLOT), np.float32)
    bpk = np.zeros((128, S * BSLOT), np.float32)
    for s, c in enumerate(slot_clusters):
        wb, bb = s * WSLOT, s * BSLOT
        we0, we1, we2 = params["w_e0"][c], params["w_e1"][c], params["w_e2"][c]
        wd0, wd1, wd2 = params["w_d0"][c], params["w_d1"][c], params["w_d2"][c]
        for k in range(KT):
            wpk[0:P, wb + _E0 + k * 256: wb + _E0 + (k + 1) * 256] = \
                we0[P * k:P * (k + 1), :]
        for k in range(2):
            wpk[0:128, wb + _E1 + 64 * k: wb + _E1 + 64 * (k + 1)] = \
                we1[128 * k:128 * (k + 1), :]
        wpk[0:64, wb + _E2: wb + _E2 + 16] = we2
        wpk[0:16, wb + _D0: wb + _D0 + 64] = wd0
        wpk[0:64, wb + _D1: wb + _D1 + 256] = wd1
        for k in range(2):
            wpk[0:128, wb + _D2 + 784 * k: wb + _D2 + 784 * (k + 1)] = \
                wd2[128 * k:128 * (k + 1), :]

        be0, be1, be2 = params["b_e0"][c], params["b_e1"][c], params["b_e2"][c]
        bd0, bd1, bd2 = params["b_d0"][c], params["b_d1"][c], params["b_d2"][c]
        bpk[0:128, bb + 0] = be0[0:128]
        bpk[0:128, bb + 1] = be0[128:256]
        bpk[0:64, bb + 2] = be1
        bpk[0:16, bb + 3] = be2
        bpk[0:64, bb + 4] = bd0
        bpk[0:128, bb + 5] = bd1[0:128]
        bpk[0:128, bb + 6] = bd1[128:256]
        for m in range(KT):
            bpk[0:P, bb + 7 + m] = bd2[P * m:P * (m + 1)]
    return wpk, bpk


def _route(labels, mode):
    """Assign rows to (core, slot) blocks; returns config + per-slot rows."""
    counts = np.bincount(labels, minlength=K)
    configs = _CONFIGS if mode == "bf16" else _CONFIGS[1:]
    for S, R in configs:
        need = int(np.sum((counts + R - 1) // R))
        if need <= N_CORES * S:
            break
    nslots = N_CORES * S
    order = np.argsort(labels, kind="stable")
    slot_cluster = np.zeros(nslots, np.int64)
    slot_rows = [np.empty(0, np.int64)] * nslots
    si = pos = 0
    for c in range(K):
        cnt = int(counts[c])
        rows_c = order[pos:pos + cnt]
        pos += cnt
        for off in range(0, cnt, R):
            slot_cluster[si] = c
            slot_rows[si] = rows_c[off:off + R]
            si += 1
    return S, R, slot_cluster, slot_rows


def _flatten_xcore(xcore_t, R, chunks):
    """[D, S*R] feature-major slab -> chunk-flattened [P, S*R*KT]."""
    ncols = xcore_t.shape[1]
    S = ncols // R
    flat = np.empty((P, ncols * KT), np.float32)
    pos = 0
    for s in range(S):
        col = s * R
        for nch in chunks:
            blk = xcore_t[:, col:col + nch]              # [784, nch]
            blk = blk.reshape(KT, P, nch).transpose(1, 0, 2)  # [P, KT, nch]
            flat[:, pos:pos + KT * nch] = blk.reshape(P, KT * nch)
            pos += KT * nch
            col += nch
    return flat


def _unflatten_ycore(yflat, R, chunks):
    """chunk-flattened [P, S*R*KT] -> row-major [S*R, D]."""
    ncols = yflat.shape[1] // KT
    S = ncols // R
    out = np.empty((ncols, D), np.float32)
    pos = 0
    for s in range(S):
        col = s * R
        for nch in chunks:
            blk = yflat[:, pos:pos + KT * nch].reshape(P, KT, nch)
            out[col:col + nch] = blk.transpose(2, 1, 0).reshape(nch, D)
            pos += KT * nch
            col += nch
    return out


def kernel_traced(inputs, trace=False, mode=None):
    if mode is None:
        mode = MODE
    x = np.ascontiguousarray(np.asarray(inputs["x"], dtype=np.float32))
    labels = np.asarray(inputs["kmeans_label"]).astype(np.int64).ravel()
    params = {k: np.asarray(v, dtype=np.float32)
              for k, v in inputs.items() if k not in ("x", "kmeans_label")}

    S, R, slot_cluster, slot_rows = _route(labels, mode)
    chunks = _chunks(R, mode)
    nc = _get_program(S, R, mode)

    in_maps = []
    for i in range(N_CORES):
        xcore = np.zeros((S * R, D), np.float32)
        for s in range(S):
            rows = slot_rows[i * S + s]
            if len(rows):
                xcore[s * R: s * R + len(rows)] = x[rows]
        wpk, bpk = _pack_weights(params, slot_cluster[i * S:(i + 1) * S])
        xflat = _flatten_xcore(np.ascontiguousarray(xcore.T), R, chunks)
        if mode == "bf16":
            import ml_dtypes
            xflat = xflat.astype(ml_dtypes.bfloat16)
            wpk = wpk.astype(ml_dtypes.bfloat16)
        in_maps.append({"xt": xflat, "wp": wpk, "bp": bpk})

    res = run_bass_kernel_spmd(nc, in_maps, core_ids=list(range(N_CORES)),
                               trace=trace)

    out = np.zeros_like(x)
    for i in range(N_CORES):
        yraw = np.asarray(res.results[i]["yt"]).astype(np.float32)
        ytT = _unflatten_ycore(yraw, R, chunks)
        for s in range(S):
            rows = slot_rows[i * S + s]
            if len(rows):
                out[rows] = ytT[s * R: s * R + len(rows)]
    return out, res


def kernel(**inputs):
    out, _ = kernel_traced(inputs, trace=False)
    return out
